# revision 1
# baseline (speedup 1.0000x reference)
"""Trainium2 Bass kernel for CFConv (gnn_message_passing).

out[n] = in_node_feat[n] * sum_{e: tgt(e)=n} filt(d_e), where filt(d) is a
function of the scalar edge distance only. The device builds a 128-point
filter table T[b] = filt(b/SCALE), then per edge does quadratic-Lagrange
interpolation in table space: the per-node sum factors into per-(node,
bucket, tap) weight histograms accumulated on the tensor engine via
per-tile one-hot matmuls, followed by one table matmul per 512 nodes.

Nodes are sharded 8 ways (6250/core); each core processes exactly the
edges targeting its nodes, so no collective is needed. Within a core,
nodes are permuted into degree-balanced groups of 8 (LPT packing) so that
every group fits exactly 2 edge tiles -- minimal padding and a fully
static, core-uniform program. Host prep does only index work (shard,
group, pad, layout); distances, table, histograms, reduction and
modulation all run on device.
"""

import sys
import numpy as np

sys.path.insert(0, "/opt/trn_rl_repo")

N = 50000
OUT_DIM = 128
NF = 64
HID = 64
NCORES = 8
NODES_PC = N // NCORES           # 6250
GROUP = 8                        # nodes per window group
NODES_PC_PAD = 6656              # 13 * 512
NGROUPS = NODES_PC_PAD // GROUP  # 832
SG_GROUPS = 512 // GROUP         # 64 groups per supergroup (512 nodes)
NSG = NODES_PC_PAD // 512        # 13
B = 128                          # table buckets
NB = 104                         # used bucket columns (covers d <= 9.7)
NTAP = 3                         # quadratic Lagrange
SLOT = 32                        # PSUM columns per group (NTAP*GROUP=24 used)
D_GRID = 12.0
SCALE = (B - 1) / D_GRID
P = 128

_cache = {}


def _lpt_groups(deg):
    """Pack NODES_PC nodes into NGROUPS groups of <= GROUP nodes, balancing
    total degree (greedy LPT). Returns [NGROUPS, GROUP] node ids (-1 pad)."""
    import heapq
    order = np.argsort(-deg, kind="stable")
    heap = [(0, g, 0) for g in range(NGROUPS)]  # (sum, group, count)
    heapq.heapify(heap)
    groups = -np.ones((NGROUPS, GROUP), np.int64)
    deferred = []
    for n in order:
        while True:
            s, g, cnt = heapq.heappop(heap)
            if cnt < GROUP:
                break
            deferred.append((s, g, cnt))
        groups[g, cnt] = n
        heapq.heappush(heap, (s + int(deg[n]), g, cnt + 1))
        for item in deferred:
            heapq.heappush(heap, item)
        deferred.clear()
    return groups


def _host_prep(inputs):
    import ml_dtypes

    pos = np.asarray(inputs["node_pos"], dtype=np.float32)
    ei = np.asarray(inputs["edge_index"])
    src = ei[0].astype(np.int64)
    tgt = ei[1].astype(np.int64)

    core = tgt // NODES_PC
    ln_all = tgt - core * NODES_PC

    per_core = []
    sizes_all = np.zeros((NCORES, NGROUPS), np.int64)
    for c in range(NCORES):
        idx = np.nonzero(core == c)[0]
        ln = ln_all[idx]
        deg = np.bincount(ln, minlength=NODES_PC)
        groups = _lpt_groups(deg)                       # [NGROUPS, GROUP]
        gsum = np.where(groups >= 0, deg[np.maximum(groups, 0)], 0).sum(axis=1)
        # sort groups by size desc so ranked sizes align across cores
        gorder = np.argsort(-gsum, kind="stable")
        groups = groups[gorder]
        gsum = gsum[gorder]
        sizes_all[c] = gsum
        # node -> (group, slot-in-group)
        n2g = np.zeros(NODES_PC, np.int64)
        n2s = np.zeros(NODES_PC, np.int64)
        valid = groups >= 0
        n2g[groups[valid]] = np.repeat(np.arange(NGROUPS), GROUP)[valid.ravel()]
        n2s[groups[valid]] = np.tile(np.arange(GROUP), NGROUPS)[valid.ravel()]
        per_core.append((idx, ln, n2g, n2s, groups))

    tiles_g = np.maximum(1, (sizes_all.max(axis=0) + P - 1) // P)
    gbase = np.zeros(NGROUPS, np.int64)
    np.cumsum(tiles_g[:-1], out=gbase[1:])
    T_TILES = int(tiles_g.sum())
    E_pad = T_TILES * P

    feats = np.asarray(inputs["in_node_feat"], dtype=np.float32)
    in_maps = []
    node_of = []                # per core: output column -> local node (-1 pad)
    for c in range(NCORES):
        idx, ln, n2g, n2s, groups = per_core[c]
        g = n2g[ln]
        slot_in_g = n2s[ln]
        order = np.argsort(g, kind="stable")
        idx = idx[order]
        g = g[order]
        slot_in_g = slot_in_g[order]
        sizes = np.bincount(g, minlength=NGROUPS)
        starts = np.zeros(NGROUPS, np.int64)
        np.cumsum(sizes[:-1], out=starts[1:])
        within = np.arange(len(idx)) - starts[g]
        slot = gbase[g] * P + within

        def plane(vals):
            a = np.zeros(E_pad, np.float32)
            a[slot] = vals
            return np.ascontiguousarray(a.reshape(T_TILES, P).T)

        s_i, t_i = src[idx], tgt[idx]
        m = {}
        m["psx"] = plane(pos[s_i, 0])
        m["psy"] = plane(pos[s_i, 1])
        m["psz"] = plane(pos[s_i, 2])
        m["ptx"] = plane(pos[t_i, 0])
        m["pty"] = plane(pos[t_i, 1])
        m["ptz"] = plane(pos[t_i, 2])

        msk = np.zeros((E_pad, GROUP), np.float32)
        msk[slot, slot_in_g] = 1.0
        msk = msk.reshape(T_TILES, P, GROUP).transpose(1, 0, 2).reshape(
            P, T_TILES * GROUP)
        m["maskS"] = np.ascontiguousarray(msk).astype(ml_dtypes.bfloat16)

        # feature columns permuted into group order
        colnode = groups.reshape(-1)                    # [NODES_PC_PAD]
        f = np.zeros((P, NODES_PC_PAD), np.float32)
        valid = colnode >= 0
        f[:, valid] = feats[c * NODES_PC + colnode[valid]].T
        m["featT"] = np.ascontiguousarray(f)
        node_of.append(colnode)
        in_maps.append(m)

    lo = float(np.asarray(inputs["lower_bound"]))
    hi = float(np.asarray(inputs["upper_bound"]))
    gamma = float(np.asarray(inputs["gamma"]))
    mu = np.linspace(lo, hi, NF, dtype=np.float32)
    W1 = np.asarray(inputs["W1"], dtype=np.float32)
    W2 = np.asarray(inputs["W2"], dtype=np.float32)
    b1 = np.asarray(inputs["b1"], dtype=np.float32)
    b2 = np.asarray(inputs["b2"], dtype=np.float32)
    consts = {
        "W1T": np.ascontiguousarray(W1.T),
        "W2T": np.ascontiguousarray(W2.T),
        "b1r": np.broadcast_to(b1, (P, HID)).copy(),
        "b2r": np.broadcast_to(b2, (P, OUT_DIM)).copy(),
        "mur": np.broadcast_to(mu, (P, NF)).copy(),
        "gridc": (np.arange(P, dtype=np.float32) / SCALE).reshape(P, 1),
        "iotaB": np.broadcast_to(
            np.arange(NB, dtype=np.float32), (P, NB)
        ).astype(ml_dtypes.bfloat16).copy(),
        "ident": np.eye(P, dtype=np.float32),
    }
    for m in in_maps:
        m.update(consts)
    return in_maps, tuple(int(x) for x in tiles_g), gamma, node_of


def _build(tiles_g, gamma):
    from concourse import bacc, mybir
    from concourse.tile import TileContext

    f32 = mybir.dt.float32
    i32 = mybir.dt.int32
    bf16 = mybir.dt.bfloat16
    AF = mybir.ActivationFunctionType
    OP = mybir.AluOpType
    LN2 = float(np.log(2.0))

    tiles_g = np.asarray(tiles_g, np.int64)
    T_TILES = int(tiles_g.sum())
    gb = np.zeros(NGROUPS + 1, np.int64)
    np.cumsum(tiles_g, out=gb[1:])
    sg_tile0 = [int(gb[SG_GROUPS * s]) for s in range(NSG)]
    sg_tiles = [int(gb[SG_GROUPS * (s + 1)] - gb[SG_GROUPS * s])
                for s in range(NSG)]
    TSG_MAX = max(sg_tiles)

    nc = bacc.Bacc("TRN2", target_bir_lowering=False, debug=False,
                   num_devices=NCORES)

    def din(name, shape, dt=f32):
        return nc.dram_tensor(name, shape, dt, kind="ExternalInput").ap()

    pos_d = {n: din(n, [P, T_TILES]) for n in
             ("psx", "psy", "psz", "ptx", "pty", "ptz")}
    maskS = din("maskS", [P, T_TILES * GROUP], bf16)
    featT = din("featT", [P, NODES_PC_PAD])
    W1T = din("W1T", [NF, HID])
    W2T = din("W2T", [HID, OUT_DIM])
    b1r = din("b1r", [P, HID])
    b2r = din("b2r", [P, OUT_DIM])
    mur = din("mur", [P, NF])
    gridc = din("gridc", [P, 1])
    iotaB = din("iotaB", [P, NB], bf16)
    ident = din("ident", [P, P])

    outT = nc.dram_tensor("outT", [P, NODES_PC_PAD], f32,
                          kind="ExternalOutput").ap()

    with TileContext(nc) as tc:
        with (
            tc.tile_pool(name="const", bufs=1) as const,
            tc.tile_pool(name="big", bufs=1) as big,
            tc.tile_pool(name="sg2", bufs=2) as sg2,
            tc.tile_pool(name="sg1", bufs=1) as sg1,
            tc.tile_pool(name="pspro", bufs=1, space="PSUM") as pspro,
            tc.tile_pool(name="psout", bufs=2, space="PSUM") as psout,
            tc.tile_pool(name="pstap", bufs=1, space="PSUM") as pstap,
        ):
            # ---------- constants ----------
            W1T_s = const.tile([NF, HID], f32, tag="w1t")
            W2T_s = const.tile([HID, OUT_DIM], f32, tag="w2t")
            b1_s = const.tile([P, HID], f32, tag="b1")
            b2_s = const.tile([P, OUT_DIM], f32, tag="b2")
            mu_s = const.tile([P, NF], f32, tag="mu")
            gr_s = const.tile([P, 1], f32, tag="gr")
            io_s = const.tile([P, NB], bf16, tag="io")
            id_s = const.tile([P, P], f32, tag="id")
            for t, d in ((W1T_s, W1T), (W2T_s, W2T), (b1_s, b1r), (b2_s, b2r),
                         (mu_s, mur), (gr_s, gridc), (io_s, iotaB),
                         (id_s, ident)):
                nc.sync.dma_start(out=t[:], in_=d)

            ln2n = const.tile([P, 1], f32, tag="ln2n")
            nc.vector.memset(ln2n[:], -LN2)
            halfc = const.tile([P, 1], f32, tag="halfc")
            nc.vector.memset(halfc[:], 0.5)
            onec = const.tile([P, 1], f32, tag="onec")
            nc.vector.memset(onec[:], 1.0)
            negonec = const.tile([P, 1], f32, tag="negonec")
            nc.vector.memset(negonec[:], -1.0)

            # ---------- filter table ----------
            tg1 = const.tile([P, NF], f32, tag="tg1")
            tg2 = const.tile([P, NF], f32, tag="tg2")
            nc.vector.tensor_tensor(out=tg1[:], in0=gr_s[:].to_broadcast([P, NF]),
                                    in1=mu_s[:], op=OP.subtract)
            nc.scalar.activation(out=tg2[:], in_=tg1[:], func=AF.Square)
            rbf = const.tile([P, NF], f32, tag="rbf")
            nc.scalar.activation(out=rbf[:], in_=tg2[:], func=AF.Exp,
                                 scale=-gamma)

            ptr1 = pspro.tile([NF, P], f32, tag="pro")
            nc.tensor.transpose(out=ptr1[:], in_=rbf[:], identity=id_s[:])
            x0t = const.tile([NF, P], f32, tag="x0t")
            nc.vector.tensor_copy(out=x0t[:], in_=ptr1[:])

            ph = pspro.tile([P, HID], f32, tag="pro")
            nc.tensor.matmul(out=ph[:], lhsT=x0t[:], rhs=W1T_s[:],
                             start=True, stop=True)
            pre1 = const.tile([P, HID], f32, tag="pre1")
            nc.vector.tensor_tensor(out=pre1[:], in0=ph[:], in1=b1_s[:],
                                    op=OP.add)
            e1 = const.tile([P, HID], f32, tag="e1")
            nc.scalar.activation(out=e1[:], in_=pre1[:], func=AF.Exp,
                                 bias=ln2n[:])
            x1 = const.tile([P, HID], f32, tag="x1")
            nc.scalar.activation(out=x1[:], in_=e1[:], func=AF.Ln,
                                 bias=halfc[:])

            ptr2 = pspro.tile([HID, P], f32, tag="pro")
            nc.tensor.transpose(out=ptr2[:], in_=x1[:], identity=id_s[:])
            x1t = const.tile([HID, P], f32, tag="x1t")
            nc.vector.tensor_copy(out=x1t[:], in_=ptr2[:])

            pf = pspro.tile([P, OUT_DIM], f32, tag="pro")
            nc.tensor.matmul(out=pf[:], lhsT=x1t[:], rhs=W2T_s[:],
                             start=True, stop=True)
            pre2 = const.tile([P, OUT_DIM], f32, tag="pre2")
            nc.vector.tensor_tensor(out=pre2[:], in0=pf[:], in1=b2_s[:],
                                    op=OP.add)
            e2 = const.tile([P, OUT_DIM], f32, tag="e2")
            nc.scalar.activation(out=e2[:], in_=pre2[:], func=AF.Exp,
                                 bias=ln2n[:])
            Tf = const.tile([P, OUT_DIM], f32, tag="Tf")
            nc.scalar.activation(out=Tf[:], in_=e2[:], func=AF.Ln,
                                 bias=halfc[:])

            Tbf = const.tile([P, OUT_DIM], bf16, tag="Tbf")
            nc.vector.tensor_copy(out=Tbf[:], in_=Tf[:])
            # shifted copies: Tsh[a][b, :] = T[b + a - 1], a in 0..2
            Tsh = []
            for a in range(NTAP):
                ts = const.tile([P, OUT_DIM], bf16, tag=f"tsh{a}")
                nc.vector.memset(ts[:], 0.0)
                o = a - 1
                if o < 0:
                    nc.sync.dma_start(out=ts[-o:P, :], in_=Tbf[0:P + o, :])
                elif o == 0:
                    nc.sync.dma_start(out=ts[:], in_=Tbf[:])
                else:
                    nc.sync.dma_start(out=ts[0:P - o, :], in_=Tbf[o:P, :])
                Tsh.append(ts)

            # ---------- edge prologue ----------
            pos_s = {}
            for tag, d in pos_d.items():
                t = big.tile([P, T_TILES], f32, tag=tag)
                nc.sync.dma_start(out=t[:], in_=d)
                pos_s[tag] = t

            s1 = big.tile([P, T_TILES], f32, tag="s1")
            s2 = big.tile([P, T_TILES], f32, tag="s2")
            acc = big.tile([P, T_TILES], f32, tag="acc")
            nc.vector.tensor_tensor(out=s1[:], in0=pos_s["psx"][:],
                                    in1=pos_s["ptx"][:], op=OP.subtract)
            nc.scalar.activation(out=acc[:], in_=s1[:], func=AF.Square)
            for cc in ("y", "z"):
                nc.vector.tensor_tensor(out=s1[:], in0=pos_s["ps" + cc][:],
                                        in1=pos_s["pt" + cc][:],
                                        op=OP.subtract)
                nc.scalar.activation(out=s2[:], in_=s1[:], func=AF.Square)
                nc.vector.tensor_tensor(out=acc[:], in0=acc[:], in1=s2[:],
                                        op=OP.add)
            # u = sqrt(d2)*SCALE, clamped so taps stay in [0, NB)
            nc.scalar.activation(out=acc[:], in_=acc[:], func=AF.Sqrt,
                                 scale=SCALE * SCALE)
            nc.vector.tensor_scalar(out=acc[:], in0=acc[:], scalar1=1.001,
                                    scalar2=NB - 2.01, op0=OP.max, op1=OP.min)
            ji = big.tile([P, T_TILES], i32, tag="ji")
            nc.scalar.copy(out=ji[:], in_=acc[:])
            nc.scalar.copy(out=s1[:], in_=ji[:])  # s1 = jf
            jbf = big.tile([P, T_TILES], bf16, tag="jbf")
            nc.scalar.copy(out=jbf[:], in_=s1[:])

            # t = u - jf in acc; taps at nodes {-1, 0, +1}
            nc.vector.tensor_tensor(out=acc[:], in0=acc[:], in1=s1[:],
                                    op=OP.subtract)
            tv = acc
            tm = big.tile([P, T_TILES], f32, tag="psx")   # reuse pos slots
            tp = big.tile([P, T_TILES], f32, tag="psy")
            tsq = big.tile([P, T_TILES], f32, tag="psz")
            nc.scalar.activation(out=tm[:], in_=tv[:], func=AF.Identity,
                                 bias=negonec[:])
            nc.scalar.activation(out=tp[:], in_=tv[:], func=AF.Identity,
                                 bias=onec[:])
            nc.scalar.activation(out=tsq[:], in_=tv[:], func=AF.Square)

            # w_-1 = t(t-1)/2 ; w_0 = 1-t^2 ; w_1 = t(t+1)/2
            wts = []
            wtmp = big.tile([P, T_TILES], f32, tag="ptx")
            for a, expr in enumerate(("m", "0", "p")):
                wb = big.tile([P, T_TILES], bf16, tag=f"w{a}")
                if expr == "m":
                    nc.vector.tensor_tensor(out=wtmp[:], in0=tv[:], in1=tm[:],
                                            op=OP.mult)
                    nc.vector.tensor_scalar(out=wb[:], in0=wtmp[:],
                                            scalar1=0.5, scalar2=None,
                                            op0=OP.mult)
                elif expr == "0":
                    nc.vector.tensor_scalar(out=wb[:], in0=tsq[:],
                                            scalar1=-1.0, scalar2=1.0,
                                            op0=OP.mult, op1=OP.add)
                else:
                    nc.vector.tensor_tensor(out=wtmp[:], in0=tv[:], in1=tp[:],
                                            op=OP.mult)
                    nc.vector.tensor_scalar(out=wb[:], in0=wtmp[:],
                                            scalar1=0.5, scalar2=None,
                                            op0=OP.mult)
                wts.append(wb)

            # ---------- main loop ----------
            for s in range(NSG):
                t0 = sg_tile0[s]
                tsg = sg_tiles[s]
                msk = sg2.tile([P, TSG_MAX * GROUP], bf16, tag="msk")
                nc.sync.dma_start(
                    out=msk[:, : tsg * GROUP],
                    in_=maskS[:, t0 * GROUP : (t0 + tsg) * GROUP])
                featc = sg2.tile([P, 512], f32, tag="featc")
                nc.sync.dma_start(out=featc[:],
                                  in_=featT[:, s * 512 : (s + 1) * 512])

                # S with 3 taps interleaved: [P, tsg, 3, GROUP]
                scat = sg1.tile([P, TSG_MAX * NTAP * GROUP], bf16, tag="scat")
                sc4 = scat[:, : tsg * NTAP * GROUP].rearrange(
                    "p (t a g) -> p t a g", a=NTAP, g=GROUP)
                mv = msk[:, : tsg * GROUP].rearrange("p (t g) -> p t g",
                                                     g=GROUP)
                for a in range(NTAP):
                    wv = wts[a][:, t0 : t0 + tsg].rearrange(
                        "p (t o) -> p t o", o=1)
                    nc.vector.tensor_tensor(
                        out=sc4[:, :, a, :],
                        in0=mv,
                        in1=wv.to_broadcast([P, tsg, GROUP]),
                        op=OP.mult,
                    )

                tap = pstap.tile([P, SG_GROUPS * SLOT], f32, tag="tap")
                tap4 = tap[:].rearrange("p (g c) -> p g c", c=SLOT)

                gt, kt = [], []
                for gl in range(SG_GROUPS):
                    cnt = int(tiles_g[SG_GROUPS * s + gl])
                    gt += [gl] * cnt
                    kt += list(range(cnt))

                nsub = (tsg + 31) // 32
                for sub in range(nsub):
                    tl = sub * 32
                    th = min(tl + 32, tsg)
                    nt = th - tl
                    Lb = sg2.tile([P, 32 * NB], bf16, tag="Lb")
                    jv = jbf[:, t0 + tl : t0 + th].rearrange(
                        "p (t o) -> p t o", o=1)
                    iov = io_s[:].rearrange("p (o b) -> p o b", o=1)
                    nc.vector.tensor_tensor(
                        out=Lb[:, : nt * NB].rearrange("p (t b) -> p t b",
                                                       b=NB),
                        in0=jv.to_broadcast([P, nt, NB]),
                        in1=iov.to_broadcast([P, nt, NB]),
                        op=OP.is_equal,
                    )
                    for tt in range(nt):
                        t_in_sg = tl + tt
                        gi = gt[t_in_sg]
                        k = kt[t_in_sg]
                        lastk = int(tiles_g[SG_GROUPS * s + gi]) - 1
                        nc.tensor.matmul(
                            out=tap4[:NB, gi, 0 : NTAP * GROUP],
                            lhsT=Lb[:, tt * NB : (tt + 1) * NB],
                            rhs=sc4[:, t_in_sg, :, :],
                            start=(k == 0),
                            stop=(k == lastk),
                        )

                outP = psout.tile([P, 512], f32, tag="outP")
                for a in range(NTAP):
                    tsb = sg2.tile([P, SG_GROUPS * GROUP], bf16, tag=f"tsb{a}")
                    nc.scalar.copy(
                        out=tsb[:NB].rearrange("p (g n) -> p g n", n=GROUP),
                        in_=tap4[:NB, :, a * GROUP : (a + 1) * GROUP],
                    )
                    nc.tensor.matmul(out=outP[:], lhsT=Tsh[a][:NB, :],
                                     rhs=tsb[:NB, :],
                                     start=(a == 0), stop=(a == NTAP - 1))

                osb = sg2.tile([P, 512], f32, tag="osb")
                nc.vector.tensor_tensor(out=osb[:], in0=outP[:], in1=featc[:],
                                        op=OP.mult)
                nc.sync.dma_start(out=outT[:, s * 512 : (s + 1) * 512],
                                  in_=osb[:])

    nc.compile()
    return nc


def kernel(**inputs):
    in_maps, tiles_g, gamma, node_of = _host_prep(inputs)

    key = (tiles_g, round(gamma, 6))
    if key not in _cache:
        _cache[key] = _build(tiles_g, gamma)
    nc = _cache[key]

    from concourse.bass_utils import run_bass_kernel_spmd

    res = run_bass_kernel_spmd(nc, in_maps, core_ids=list(range(NCORES)))

    out = np.empty((N, OUT_DIM), np.float32)
    for c in range(NCORES):
        colnode = node_of[c]
        valid = colnode >= 0
        out[c * NODES_PC + colnode[valid]] = \
            res.results[c]["outT"][:, valid].T
    return out



# revision 7
# speedup vs baseline: 2.7622x; 2.7622x over previous
"""Trainium2 Bass kernel for CFConv (gnn_message_passing).

out[n] = in_node_feat[n] * sum_{e: tgt(e)=n} filt(d_e), where filt(d) is a
function of the scalar edge distance only. The device builds a 64-point
filter table T[b] = filt(b*h) plus the precombined derivative tables
T'_0 = T, T'_1 = (T[b+1]-T[b-1])/2, T'_2 = (T[b+1]+T[b-1])/2 - T[b], so the
quadratic-Lagrange interpolation f(u) = T'_0[j] + t*T'_1[j] + t^2*T'_2[j]
(u = d/h, j = int(u), t = u - j) factors into three per-(node, bucket)
moment histograms H_m[b, n] = sum_{e->n} delta(j_e, b) * t_e^m, accumulated
on the tensor engine, followed by 3 table matmuls per 512 nodes.

The bucket one-hot is built bucket-major (Lb[p, b, t]) with one
tensor_scalar(is_equal, const b) per bucket row, which runs in the DVE
4x perf mode (all-SBUF packed bf16), in 4 big tile-chunks to amortize
per-instruction overheads. The moment rhs is [mask, mask*t, mask*t^2]
with the mask DMA'd straight into its slot and the two products running
in the DVE 2x mode.

Nodes are sharded 8 ways (6250/core); each core processes exactly the
edges targeting its nodes, so no collective is needed. Within a core,
nodes are permuted into degree-balanced groups of 8 (LPT packing) so that
every group fits exactly 2 edge tiles. Host prep does only index work
(shard, group, pad, layout); distances, table, histograms, reduction and
modulation all run on device.
"""

import sys
import numpy as np

sys.path.insert(0, "/opt/trn_rl_repo")

N = 50000
OUT_DIM = 128
NF = 64
HID = 64
NCORES = 8
NODES_PC = N // NCORES           # 6250
GROUP = 8                        # nodes per window group
NODES_PC_PAD = 6656              # 13 * 512
NGROUPS = NODES_PC_PAD // GROUP  # 832
SG_GROUPS = 512 // GROUP         # 64 groups per supergroup (512 nodes)
NSG = NODES_PC_PAD // 512        # 13
NB = 64                          # table buckets
NM = 3                           # moments 1, t, t^2
DMAX = 8.5                       # table covers d in [0, DMAX]
SCALE = (NB - 1) / DMAX
NCH = 4                          # edge-tile chunks for the one-hot build
P = 128

_cache = {}


def _lpt_groups(deg):
    """Pack NODES_PC nodes into NGROUPS groups of <= GROUP nodes, balancing
    total degree (greedy LPT). Returns [NGROUPS, GROUP] node ids (-1 pad)."""
    import heapq
    order = np.argsort(-deg, kind="stable")
    heap = [(0, g, 0) for g in range(NGROUPS)]  # (sum, group, count)
    heapq.heapify(heap)
    groups = -np.ones((NGROUPS, GROUP), np.int64)
    deferred = []
    for n in order:
        while True:
            s, g, cnt = heapq.heappop(heap)
            if cnt < GROUP:
                break
            deferred.append((s, g, cnt))
        groups[g, cnt] = n
        heapq.heappush(heap, (s + int(deg[n]), g, cnt + 1))
        for item in deferred:
            heapq.heappush(heap, item)
        deferred.clear()
    return groups


def _host_prep(inputs):
    import ml_dtypes

    pos = np.asarray(inputs["node_pos"], dtype=np.float32)
    ei = np.asarray(inputs["edge_index"])
    src = ei[0].astype(np.int64)
    tgt = ei[1].astype(np.int64)

    core = tgt // NODES_PC
    ln_all = tgt - core * NODES_PC

    per_core = []
    sizes_all = np.zeros((NCORES, NGROUPS), np.int64)
    for c in range(NCORES):
        idx = np.nonzero(core == c)[0]
        ln = ln_all[idx]
        deg = np.bincount(ln, minlength=NODES_PC)
        groups = _lpt_groups(deg)                       # [NGROUPS, GROUP]
        gsum = np.where(groups >= 0, deg[np.maximum(groups, 0)], 0).sum(axis=1)
        # sort groups by size desc so ranked sizes align across cores
        gorder = np.argsort(-gsum, kind="stable")
        groups = groups[gorder]
        gsum = gsum[gorder]
        sizes_all[c] = gsum
        # node -> (group, slot-in-group)
        n2g = np.zeros(NODES_PC, np.int64)
        n2s = np.zeros(NODES_PC, np.int64)
        valid = groups >= 0
        n2g[groups[valid]] = np.repeat(np.arange(NGROUPS), GROUP)[valid.ravel()]
        n2s[groups[valid]] = np.tile(np.arange(GROUP), NGROUPS)[valid.ravel()]
        per_core.append((idx, ln, n2g, n2s, groups))

    tiles_g = np.maximum(1, (sizes_all.max(axis=0) + P - 1) // P)
    gbase = np.zeros(NGROUPS, np.int64)
    np.cumsum(tiles_g[:-1], out=gbase[1:])
    T_TILES = int(tiles_g.sum())
    E_pad = T_TILES * P

    feats = np.asarray(inputs["in_node_feat"], dtype=np.float32)
    in_maps = []
    node_of = []                # per core: output column -> local node (-1 pad)
    for c in range(NCORES):
        idx, ln, n2g, n2s, groups = per_core[c]
        g = n2g[ln]
        slot_in_g = n2s[ln]
        order = np.argsort(g, kind="stable")
        idx = idx[order]
        g = g[order]
        slot_in_g = slot_in_g[order]
        sizes = np.bincount(g, minlength=NGROUPS)
        starts = np.zeros(NGROUPS, np.int64)
        np.cumsum(sizes[:-1], out=starts[1:])
        within = np.arange(len(idx)) - starts[g]
        slot = gbase[g] * P + within

        def plane(vals):
            a = np.zeros(E_pad, np.float32)
            a[slot] = vals
            return np.ascontiguousarray(a.reshape(T_TILES, P).T)

        s_i, t_i = src[idx], tgt[idx]
        m = {}
        # all six planes packed into one [P, 6*T_TILES] tensor
        m["posP"] = np.ascontiguousarray(np.concatenate(
            [plane(pos[s_i, 0]), plane(pos[s_i, 1]), plane(pos[s_i, 2]),
             plane(pos[t_i, 0]), plane(pos[t_i, 1]), plane(pos[t_i, 2])],
            axis=1))

        # mask in group-major layout [P, GROUP, T_TILES]
        msk = np.zeros((E_pad, GROUP), np.float32)
        msk[slot, slot_in_g] = 1.0
        msk = msk.reshape(T_TILES, P, GROUP).transpose(1, 2, 0).reshape(
            P, GROUP * T_TILES)
        m["maskS"] = np.ascontiguousarray(msk).astype(ml_dtypes.bfloat16)

        # feature columns permuted into group order
        colnode = groups.reshape(-1)                    # [NODES_PC_PAD]
        f = np.zeros((P, NODES_PC_PAD), np.float32)
        valid = colnode >= 0
        f[:, valid] = feats[c * NODES_PC + colnode[valid]].T
        m["featT"] = np.ascontiguousarray(f)
        node_of.append(colnode)
        in_maps.append(m)

    lo = float(np.asarray(inputs["lower_bound"]))
    hi = float(np.asarray(inputs["upper_bound"]))
    gamma = float(np.asarray(inputs["gamma"]))
    mu = np.linspace(lo, hi, NF, dtype=np.float32)
    W1 = np.asarray(inputs["W1"], dtype=np.float32)
    W2 = np.asarray(inputs["W2"], dtype=np.float32)
    b1 = np.asarray(inputs["b1"], dtype=np.float32)
    b2 = np.asarray(inputs["b2"], dtype=np.float32)
    consts = {
        "W1T": np.ascontiguousarray(W1.T),
        "W2T": np.ascontiguousarray(W2.T),
        "b1r": np.broadcast_to(b1, (P, HID)).copy(),
        "b2r": np.broadcast_to(b2, (P, OUT_DIM)).copy(),
        "mur": np.broadcast_to(mu, (P, NF)).copy(),
        "gridc": (np.arange(P, dtype=np.float32) / SCALE).reshape(P, 1),
        "ident": np.eye(P, dtype=np.float32),
    }
    for m in in_maps:
        m.update(consts)
    return in_maps, tuple(int(x) for x in tiles_g), gamma, node_of


def _build(tiles_g, gamma):
    from concourse import bacc, mybir
    from concourse.tile import TileContext

    f32 = mybir.dt.float32
    f32r = mybir.dt.float32r
    i32 = mybir.dt.int32
    bf16 = mybir.dt.bfloat16
    AF = mybir.ActivationFunctionType
    OP = mybir.AluOpType
    LN2 = float(np.log(2.0))

    tiles_g = np.asarray(tiles_g, np.int64)
    T_TILES = int(tiles_g.sum())
    gb = np.zeros(NGROUPS + 1, np.int64)
    np.cumsum(tiles_g, out=gb[1:])
    sg_tile0 = [int(gb[SG_GROUPS * s]) for s in range(NSG)]
    sg_tend = [int(gb[SG_GROUPS * (s + 1)]) for s in range(NSG)]

    # chunk bounds (even split)
    cb = [int(round(T_TILES * i / NCH)) for i in range(NCH + 1)]
    T_CH = max(cb[i + 1] - cb[i] for i in range(NCH))
    # sgs consumed after each chunk
    sg_of_chunk = [[] for _ in range(NCH)]
    for s in range(NSG):
        for c in range(NCH):
            if sg_tend[s] <= cb[c + 1]:
                sg_of_chunk[c].append(s)
                break

    nc = bacc.Bacc("TRN2", target_bir_lowering=False, debug=False,
                   num_devices=NCORES)

    def din(name, shape, dt=f32):
        return nc.dram_tensor(name, shape, dt, kind="ExternalInput").ap()

    posP = din("posP", [P, 6 * T_TILES])
    maskS = din("maskS", [P, GROUP * T_TILES], bf16)
    featT = din("featT", [P, NODES_PC_PAD])
    W1T = din("W1T", [NF, HID])
    W2T = din("W2T", [HID, OUT_DIM])
    b1r = din("b1r", [P, HID])
    b2r = din("b2r", [P, OUT_DIM])
    mur = din("mur", [P, NF])
    gridc = din("gridc", [P, 1])
    ident = din("ident", [P, P])

    outT = nc.dram_tensor("outT", [P, NODES_PC_PAD], f32,
                          kind="ExternalOutput").ap()

    with TileContext(nc) as tc:
        with (
            tc.tile_pool(name="const", bufs=1) as const,
            tc.tile_pool(name="chpos", bufs=2) as chpos,
            tc.tile_pool(name="chwork", bufs=2) as chwork,
            tc.tile_pool(name="chlb", bufs=2) as chlb,
            tc.tile_pool(name="chsc", bufs=2) as chsc,
            tc.tile_pool(name="sgp", bufs=2) as sgp,
            tc.tile_pool(name="pspro", bufs=1, space="PSUM") as pspro,
            tc.tile_pool(name="pstap", bufs=2, space="PSUM") as pstap,
            tc.tile_pool(name="psout", bufs=1, space="PSUM") as psout,
        ):
            # ---------- constants ----------
            W1T_s = const.tile([NF, HID], f32, tag="w1t")
            W2T_s = const.tile([HID, OUT_DIM], f32, tag="w2t")
            b1_s = const.tile([P, HID], f32, tag="b1")
            b2_s = const.tile([P, OUT_DIM], f32, tag="b2")
            mu_s = const.tile([P, NF], f32, tag="mu")
            gr_s = const.tile([P, 1], f32, tag="gr")
            id_s = const.tile([P, P], f32, tag="id")
            for t, d in ((W1T_s, W1T), (W2T_s, W2T), (b1_s, b1r), (b2_s, b2r),
                         (mu_s, mur), (gr_s, gridc), (id_s, ident)):
                nc.sync.dma_start(out=t[:], in_=d)

            ln2n = const.tile([P, 1], f32, tag="ln2n")
            nc.vector.memset(ln2n[:], -LN2)
            halfc = const.tile([P, 1], f32, tag="halfc")
            nc.vector.memset(halfc[:], 0.5)

            # ---------- filter table (rows 0..P-1; only 0..NB+1 matter) ----
            tg1 = const.tile([P, NF], f32, tag="tg1")
            tg2 = const.tile([P, NF], f32, tag="tg2")
            nc.vector.tensor_tensor(out=tg1[:], in0=gr_s[:].to_broadcast([P, NF]),
                                    in1=mu_s[:], op=OP.subtract)
            nc.scalar.activation(out=tg2[:], in_=tg1[:], func=AF.Square)
            rbf = const.tile([P, NF], f32, tag="rbf")
            nc.scalar.activation(out=rbf[:], in_=tg2[:], func=AF.Exp,
                                 scale=-gamma)

            ptr1 = pspro.tile([NF, P], f32, tag="pro")
            nc.tensor.transpose(out=ptr1[:], in_=rbf[:], identity=id_s[:])
            x0t = const.tile([NF, P], f32, tag="x0t")
            nc.vector.tensor_copy(out=x0t[:], in_=ptr1[:])

            ph = pspro.tile([P, HID], f32, tag="pro")
            nc.tensor.matmul(out=ph[:], lhsT=x0t[:], rhs=W1T_s[:],
                             start=True, stop=True)
            pre1 = const.tile([P, HID], f32, tag="pre1")
            nc.vector.tensor_tensor(out=pre1[:], in0=ph[:], in1=b1_s[:],
                                    op=OP.add)
            e1 = const.tile([P, HID], f32, tag="e1")
            nc.scalar.activation(out=e1[:], in_=pre1[:], func=AF.Exp,
                                 bias=ln2n[:])
            x1 = const.tile([P, HID], f32, tag="x1")
            nc.scalar.activation(out=x1[:], in_=e1[:], func=AF.Ln,
                                 bias=halfc[:])

            ptr2 = pspro.tile([HID, P], f32, tag="pro")
            nc.tensor.transpose(out=ptr2[:], in_=x1[:], identity=id_s[:])
            x1t = const.tile([HID, P], f32, tag="x1t")
            nc.vector.tensor_copy(out=x1t[:], in_=ptr2[:])

            pf = pspro.tile([P, OUT_DIM], f32, tag="pro")
            nc.tensor.matmul(out=pf[:], lhsT=x1t[:], rhs=W2T_s[:],
                             start=True, stop=True)
            pre2 = const.tile([P, OUT_DIM], f32, tag="pre2")
            nc.vector.tensor_tensor(out=pre2[:], in0=pf[:], in1=b2_s[:],
                                    op=OP.add)
            e2 = const.tile([P, OUT_DIM], f32, tag="e2")
            nc.scalar.activation(out=e2[:], in_=pre2[:], func=AF.Exp,
                                 bias=ln2n[:])
            Tf = const.tile([P, OUT_DIM], f32r, tag="Tf")
            nc.scalar.activation(out=Tf[:], in_=e2[:], func=AF.Ln,
                                 bias=halfc[:])

            # shifted copies on bucket rows 0..NB-1
            Tp_s = const.tile([NB, OUT_DIM], f32r, tag="tp")
            Tm_s = const.tile([NB, OUT_DIM], f32r, tag="tm")
            nc.sync.dma_start(out=Tp_s[:], in_=Tf[1:NB + 1, :])
            # row 0 pairs only with bucket 0, which never fires (u >= 1):
            # fill it with T[0] instead of memset (no f32r memset in ISA)
            nc.sync.dma_start(out=Tm_s[0:1, :], in_=Tf[0:1, :])
            nc.sync.dma_start(out=Tm_s[1:NB, :], in_=Tf[0:NB - 1, :])

            # T'_1 = (Tp - Tm)/2 ; T'_2 = (Tp + Tm)/2 - T
            T1b = const.tile([NB, OUT_DIM], f32r, tag="t1b")
            T2b = const.tile([NB, OUT_DIM], f32r, tag="t2b")
            ttmp = const.tile([NB, OUT_DIM], f32r, tag="ttmp")
            nc.vector.tensor_tensor(out=ttmp[:], in0=Tp_s[:], in1=Tm_s[:],
                                    op=OP.subtract)
            nc.vector.tensor_scalar(out=T1b[:], in0=ttmp[:], scalar1=0.5,
                                    scalar2=None, op0=OP.mult)
            nc.vector.tensor_tensor(out=ttmp[:], in0=Tp_s[:], in1=Tm_s[:],
                                    op=OP.add)
            nc.vector.tensor_scalar(out=ttmp[:], in0=ttmp[:], scalar1=0.5,
                                    scalar2=None, op0=OP.mult)
            nc.vector.tensor_tensor(out=T2b[:], in0=ttmp[:], in1=Tf[0:NB, :],
                                    op=OP.subtract)
            Ttabs = [Tf, T1b, T2b]   # lhsT for moments 0,1,2 (rows 0..NB-1)

            posv = posP.rearrange("p (k t) -> p k t", k=6)
            maskv = maskS.rearrange("p (g t) -> p g t", g=GROUP)

            lb_tiles = {}
            sc_tiles = {}

            # tile -> (supergroup-local group, k, lastk)
            tinfo = []
            for g in range(NGROUPS):
                cnt = int(tiles_g[g])
                for k in range(cnt):
                    tinfo.append((g % SG_GROUPS, k, cnt - 1))

            for c in range(NCH):
                c0, c1 = cb[c], cb[c + 1]
                tl = c1 - c0

                # ---------- chunk build ----------
                post = chpos.tile([P, 6 * T_CH], f32, tag="post")
                pov = post[:].rearrange("p (k t) -> p k t", k=6)
                nc.sync.dma_start(out=pov[:, :, :tl], in_=posv[:, :, c0:c1])

                scat = chsc.tile([P, NM * GROUP * T_CH], bf16, tag="scat")
                scv = scat[:].rearrange("p (m g t) -> p m g t", m=NM, g=GROUP)
                nc.sync.dma_start(out=scv[:, 0, :, :tl],
                                  in_=maskv[:, :, c0:c1])

                w1 = chwork.tile([P, T_CH], f32, tag="w1")
                s2 = chwork.tile([P, T_CH], f32, tag="s2")
                acc = chwork.tile([P, T_CH], f32, tag="acc")
                nc.gpsimd.tensor_tensor(out=w1[:, :tl], in0=pov[:, 0, :tl],
                                        in1=pov[:, 3, :tl], op=OP.subtract)
                nc.scalar.activation(out=acc[:, :tl], in_=w1[:, :tl],
                                     func=AF.Square)
                nc.gpsimd.tensor_tensor(out=w1[:, :tl], in0=pov[:, 1, :tl],
                                        in1=pov[:, 4, :tl], op=OP.subtract)
                nc.scalar.activation(out=s2[:, :tl], in_=w1[:, :tl],
                                     func=AF.Square)
                nc.gpsimd.tensor_tensor(out=acc[:, :tl], in0=acc[:, :tl],
                                        in1=s2[:, :tl], op=OP.add)
                nc.gpsimd.tensor_tensor(out=w1[:, :tl], in0=pov[:, 2, :tl],
                                        in1=pov[:, 5, :tl], op=OP.subtract)
                nc.scalar.activation(out=s2[:, :tl], in_=w1[:, :tl],
                                     func=AF.Square)
                nc.gpsimd.tensor_tensor(out=acc[:, :tl], in0=acc[:, :tl],
                                        in1=s2[:, :tl], op=OP.add)
                # u = sqrt(d2)*SCALE, clamped so taps stay in [0, NB)
                nc.scalar.activation(out=acc[:, :tl], in_=acc[:, :tl],
                                     func=AF.Sqrt, scale=SCALE * SCALE)
                nc.vector.tensor_scalar(out=acc[:, :tl], in0=acc[:, :tl],
                                        scalar1=1.001, scalar2=NB - 2.01,
                                        op0=OP.max, op1=OP.min)
                ji = chwork.tile([P, T_CH], i32, tag="ji")
                nc.scalar.copy(out=ji[:, :tl], in_=acc[:, :tl])
                jb = chwork.tile([P, T_CH], bf16, tag="jb")
                nc.scalar.copy(out=jb[:, :tl], in_=ji[:, :tl])
                tb = chwork.tile([P, T_CH], bf16, tag="tb")
                nc.vector.tensor_tensor(out=tb[:, :tl], in0=acc[:, :tl],
                                        in1=jb[:, :tl], op=OP.subtract)

                # one-hot rows (4x DVE mode: all-SBUF packed bf16)
                lb = chlb.tile([P, NB * T_CH], bf16, tag="lb")
                lbv = lb[:].rearrange("p (b t) -> p b t", b=NB)
                for b in range(NB):
                    nc.vector.tensor_scalar(out=lbv[:, b, :tl],
                                            in0=jb[:, :tl],
                                            scalar1=float(b), scalar2=None,
                                            op0=OP.is_equal)

                # moment planes: m1 = mask*t, m2 = m1*t  (2x DVE mode)
                tbv = tb[:, :tl].rearrange("p (o t) -> p o t", o=1)
                nc.vector.tensor_tensor(
                    out=scv[:, 1, :, :tl], in0=scv[:, 0, :, :tl],
                    in1=tbv.to_broadcast([P, GROUP, tl]), op=OP.mult)
                nc.vector.tensor_tensor(
                    out=scv[:, 2, :, :tl], in0=scv[:, 1, :, :tl],
                    in1=tbv.to_broadcast([P, GROUP, tl]), op=OP.mult)

                lb_tiles[c] = (lbv, c0)
                sc_tiles[c] = (scv, c0)

                # ---------- consume finished supergroups ----------
                for s in sg_of_chunk[c]:
                    featc = sgp.tile([P, 512], f32, tag="featc")
                    nc.sync.dma_start(out=featc[:],
                                      in_=featT[:, s * 512:(s + 1) * 512])

                    # two PSUM tiles so no matmul output crosses a 2KB
                    # PSUM bank boundary (group blocks of 32B / 64B)
                    tap0 = pstap.tile([NB, SG_GROUPS * GROUP], f32,
                                      tag="tap0")
                    t0v = tap0[:].rearrange("p (g q) -> p g q", q=GROUP)
                    tap12 = pstap.tile([NB, SG_GROUPS * 2 * GROUP], f32,
                                       tag="tap12")
                    t12v = tap12[:].rearrange("p (g m q) -> p g m q", m=2,
                                              q=GROUP)
                    for tt in range(sg_tile0[s], sg_tend[s]):
                        gl, k, lastk = tinfo[tt]
                        cc = c if tt >= cb[c] else c - 1
                        lbv_c, lc0 = lb_tiles[cc]
                        scv_c, sc0 = sc_tiles[cc]
                        nc.tensor.matmul(
                            out=t0v[:, gl, :],
                            lhsT=lbv_c[:, :, tt - lc0],
                            rhs=scv_c[:, 0, :, tt - sc0],
                            start=(k == 0), stop=(k == lastk))
                        nc.tensor.matmul(
                            out=t12v[:, gl, :, :],
                            lhsT=lbv_c[:, :, tt - lc0],
                            rhs=scv_c[:, 1:3, :, tt - sc0],
                            start=(k == 0), stop=(k == lastk))

                    outP = psout.tile([P, 512], f32, tag="outP")
                    for m in range(NM):
                        tsb = sgp.tile([NB, 512], f32r, tag=f"tsb{m}")
                        if m == 0:
                            nc.scalar.copy(out=tsb[:], in_=t0v[:, :, :])
                        else:
                            nc.scalar.copy(out=tsb[:],
                                           in_=t12v[:, :, m - 1, :])
                        nc.tensor.matmul(out=outP[:], lhsT=Ttabs[m][0:NB, :],
                                         rhs=tsb[:], start=(m == 0),
                                         stop=(m == NM - 1))

                    osb = sgp.tile([P, 512], f32, tag="osb")
                    nc.vector.tensor_tensor(out=osb[:], in0=outP[:],
                                            in1=featc[:], op=OP.mult)
                    nc.sync.dma_start(out=outT[:, s * 512:(s + 1) * 512],
                                      in_=osb[:])

    nc.compile()
    return nc


def kernel(**inputs):
    in_maps, tiles_g, gamma, node_of = _host_prep(inputs)

    key = (tiles_g, round(gamma, 6))
    if key not in _cache:
        _cache[key] = _build(tiles_g, gamma)
    nc = _cache[key]

    from concourse.bass_utils import run_bass_kernel_spmd

    res = run_bass_kernel_spmd(nc, in_maps, core_ids=list(range(NCORES)))

    out = np.empty((N, OUT_DIM), np.float32)
    for c in range(NCORES):
        colnode = node_of[c]
        valid = colnode >= 0
        out[c * NODES_PC + colnode[valid]] = \
            res.results[c]["outT"][:, valid].T
    return out


# revision 16
# speedup vs baseline: 3.1640x; 1.1455x over previous
"""Trainium2 Bass kernel for CFConv (gnn_message_passing).

out[n] = in_node_feat[n] * sum_{e: tgt(e)=n} filt(d_e), where filt(d) is a
function of the scalar edge distance only. The device builds a 64-point
filter table T[b] = filt(b*h) plus the precombined derivative tables
T'_0 = T, T'_1 = (T[b+1]-T[b-1])/2, T'_2 = (T[b+1]+T[b-1])/2 - T[b], so the
quadratic-Lagrange interpolation f(u) = T'_0[j] + t*T'_1[j] + t^2*T'_2[j]
(u = d/h, j = int(u), t = u - j) factors into three per-(node, bucket)
moment histograms H_m[b, n] = sum_{e->n} delta(j_e, b) * t_e^m, accumulated
on the tensor engine, followed by 3 table matmuls per 512 nodes.

The bucket one-hot is built bucket-major (Lb[p, b, t]) with one
tensor_scalar(is_equal, const b) per bucket row, which runs in the DVE
4x perf mode (all-SBUF packed bf16), in 4 big tile-chunks to amortize
per-instruction overheads. The moment rhs is [mask, mask*t, mask*t^2]
with the mask DMA'd straight into its slot and the two products running
in the DVE 2x mode.

Nodes are sharded 8 ways (6250/core); each core processes exactly the
edges targeting its nodes, so no collective is needed. Within a core,
nodes are permuted into degree-balanced groups of 8 (LPT packing) so that
every group fits exactly 2 edge tiles. Host prep does only index work
(shard, group, pad, layout); distances, table, histograms, reduction and
modulation all run on device.
"""

import sys
import numpy as np

sys.path.insert(0, "/opt/trn_rl_repo")

N = 50000
OUT_DIM = 128
NF = 64
HID = 64
NCORES = 8
NODES_PC = N // NCORES           # 6250
GROUP = 8                        # nodes per window group
NODES_PC_PAD = 6656              # 13 * 512
NGROUPS = NODES_PC_PAD // GROUP  # 832
SG_GROUPS = 512 // GROUP         # 64 groups per supergroup (512 nodes)
NSG = NODES_PC_PAD // 512        # 13
NB = 64                          # table buckets
NM = 3                           # moments 1, t, t^2
DMAX = 8.5                       # table covers d in [0, DMAX]
SCALE = (NB - 1) / DMAX
NCH = 4                          # edge-tile chunks for the one-hot build
P = 128

_cache = {}


def _lpt_groups(deg):
    """Pack NODES_PC nodes into NGROUPS groups of <= GROUP nodes, balancing
    total degree (greedy LPT). Returns [NGROUPS, GROUP] node ids (-1 pad)."""
    import heapq
    order = np.argsort(-deg, kind="stable")
    heap = [(0, g, 0) for g in range(NGROUPS)]  # (sum, group, count)
    heapq.heapify(heap)
    groups = -np.ones((NGROUPS, GROUP), np.int64)
    deferred = []
    for n in order:
        while True:
            s, g, cnt = heapq.heappop(heap)
            if cnt < GROUP:
                break
            deferred.append((s, g, cnt))
        groups[g, cnt] = n
        heapq.heappush(heap, (s + int(deg[n]), g, cnt + 1))
        for item in deferred:
            heapq.heappush(heap, item)
        deferred.clear()
    return groups


def _host_prep(inputs):
    import ml_dtypes

    pos = np.asarray(inputs["node_pos"], dtype=np.float32)
    ei = np.asarray(inputs["edge_index"])
    src = ei[0].astype(np.int64)
    tgt = ei[1].astype(np.int64)

    core = tgt // NODES_PC
    ln_all = tgt - core * NODES_PC

    per_core = []
    sizes_all = np.zeros((NCORES, NGROUPS), np.int64)
    for c in range(NCORES):
        idx = np.nonzero(core == c)[0]
        ln = ln_all[idx]
        deg = np.bincount(ln, minlength=NODES_PC)
        groups = _lpt_groups(deg)                       # [NGROUPS, GROUP]
        gsum = np.where(groups >= 0, deg[np.maximum(groups, 0)], 0).sum(axis=1)
        # sort groups by size desc so ranked sizes align across cores
        gorder = np.argsort(-gsum, kind="stable")
        groups = groups[gorder]
        gsum = gsum[gorder]
        sizes_all[c] = gsum
        # node -> (group, slot-in-group)
        n2g = np.zeros(NODES_PC, np.int64)
        n2s = np.zeros(NODES_PC, np.int64)
        valid = groups >= 0
        n2g[groups[valid]] = np.repeat(np.arange(NGROUPS), GROUP)[valid.ravel()]
        n2s[groups[valid]] = np.tile(np.arange(GROUP), NGROUPS)[valid.ravel()]
        per_core.append((idx, ln, n2g, n2s, groups))

    tiles_g = np.maximum(1, (sizes_all.max(axis=0) + P - 1) // P)
    gbase = np.zeros(NGROUPS, np.int64)
    np.cumsum(tiles_g[:-1], out=gbase[1:])
    T_TILES = int(tiles_g.sum())
    E_pad = T_TILES * P

    feats = np.asarray(inputs["in_node_feat"], dtype=np.float32)
    in_maps = []
    node_of = []                # per core: output column -> local node (-1 pad)
    for c in range(NCORES):
        idx, ln, n2g, n2s, groups = per_core[c]
        g = n2g[ln]
        slot_in_g = n2s[ln]
        order = np.argsort(g, kind="stable")
        idx = idx[order]
        g = g[order]
        slot_in_g = slot_in_g[order]
        sizes = np.bincount(g, minlength=NGROUPS)
        starts = np.zeros(NGROUPS, np.int64)
        np.cumsum(sizes[:-1], out=starts[1:])
        within = np.arange(len(idx)) - starts[g]
        slot = gbase[g] * P + within

        def plane(vals):
            a = np.zeros(E_pad, np.float32)
            a[slot] = vals
            return np.ascontiguousarray(a.reshape(T_TILES, P).T)

        s_i, t_i = src[idx], tgt[idx]
        m = {}
        # all six planes packed into one [P, 6*T_TILES] tensor, (src,tgt)
        # pairs adjacent so per-axis DMAs can feed the subtractions early
        m["posP"] = np.ascontiguousarray(np.concatenate(
            [plane(pos[s_i, 0]), plane(pos[t_i, 0]), plane(pos[s_i, 1]),
             plane(pos[t_i, 1]), plane(pos[s_i, 2]), plane(pos[t_i, 2])],
            axis=1))

        # mask in group-major layout [P, GROUP, T_TILES]
        msk = np.zeros((E_pad, GROUP), np.float32)
        msk[slot, slot_in_g] = 1.0
        msk = msk.reshape(T_TILES, P, GROUP).transpose(1, 2, 0).reshape(
            P, GROUP * T_TILES)
        m["maskS"] = np.ascontiguousarray(msk).astype(ml_dtypes.bfloat16)

        # feature columns permuted into group order
        colnode = groups.reshape(-1)                    # [NODES_PC_PAD]
        f = np.zeros((P, NODES_PC_PAD), np.float32)
        valid = colnode >= 0
        f[:, valid] = feats[c * NODES_PC + colnode[valid]].T
        m["featT"] = np.ascontiguousarray(f)
        node_of.append(colnode)
        in_maps.append(m)

    lo = float(np.asarray(inputs["lower_bound"]))
    hi = float(np.asarray(inputs["upper_bound"]))
    gamma = float(np.asarray(inputs["gamma"]))
    mu = np.linspace(lo, hi, NF, dtype=np.float32)
    W1 = np.asarray(inputs["W1"], dtype=np.float32)
    W2 = np.asarray(inputs["W2"], dtype=np.float32)
    b1 = np.asarray(inputs["b1"], dtype=np.float32)
    b2 = np.asarray(inputs["b2"], dtype=np.float32)
    consts = {
        "W1T": np.ascontiguousarray(W1.T),
        "W2T": np.ascontiguousarray(W2.T),
        "b1r": np.broadcast_to(b1, (P, HID)).copy(),
        "b2r": np.broadcast_to(b2, (P, OUT_DIM)).copy(),
        "mur": np.broadcast_to(mu, (P, NF)).copy(),
        "gridc": (np.arange(P, dtype=np.float32) / SCALE).reshape(P, 1),
        "ident": np.eye(P, dtype=np.float32),
    }
    for m in in_maps:
        m.update(consts)
    return in_maps, tuple(int(x) for x in tiles_g), gamma, node_of


def _build(tiles_g, gamma):
    from concourse import bacc, mybir
    from concourse.tile import TileContext

    f32 = mybir.dt.float32
    f32r = mybir.dt.float32r
    i32 = mybir.dt.int32
    bf16 = mybir.dt.bfloat16
    AF = mybir.ActivationFunctionType
    OP = mybir.AluOpType
    LN2 = float(np.log(2.0))

    tiles_g = np.asarray(tiles_g, np.int64)
    T_TILES = int(tiles_g.sum())
    gb = np.zeros(NGROUPS + 1, np.int64)
    np.cumsum(tiles_g, out=gb[1:])
    sg_tile0 = [int(gb[SG_GROUPS * s]) for s in range(NSG)]
    sg_tend = [int(gb[SG_GROUPS * (s + 1)]) for s in range(NSG)]

    # chunk bounds (even split)
    cb = [int(round(T_TILES * i / NCH)) for i in range(NCH + 1)]
    T_CH = max(cb[i + 1] - cb[i] for i in range(NCH))
    # sgs consumed after each chunk
    sg_of_chunk = [[] for _ in range(NCH)]
    for s in range(NSG):
        for c in range(NCH):
            if sg_tend[s] <= cb[c + 1]:
                sg_of_chunk[c].append(s)
                break

    nc = bacc.Bacc("TRN2", target_bir_lowering=False, debug=False,
                   num_devices=NCORES)

    def din(name, shape, dt=f32):
        return nc.dram_tensor(name, shape, dt, kind="ExternalInput").ap()

    posP = din("posP", [P, 6 * T_TILES])
    maskS = din("maskS", [P, GROUP * T_TILES], bf16)
    featT = din("featT", [P, NODES_PC_PAD])
    W1T = din("W1T", [NF, HID])
    W2T = din("W2T", [HID, OUT_DIM])
    b1r = din("b1r", [P, HID])
    b2r = din("b2r", [P, OUT_DIM])
    mur = din("mur", [P, NF])
    gridc = din("gridc", [P, 1])
    ident = din("ident", [P, P])

    outT = nc.dram_tensor("outT", [P, NODES_PC_PAD], f32,
                          kind="ExternalOutput").ap()

    with TileContext(nc) as tc:
        with (
            tc.tile_pool(name="const", bufs=1) as const,
            tc.tile_pool(name="chpos", bufs=2) as chpos,
            tc.tile_pool(name="chwork", bufs=2) as chwork,
            tc.tile_pool(name="chlb", bufs=2) as chlb,
            tc.tile_pool(name="chsc", bufs=2) as chsc,
            tc.tile_pool(name="sgp", bufs=2) as sgp,
            tc.tile_pool(name="pspro", bufs=1, space="PSUM") as pspro,
            tc.tile_pool(name="pstap", bufs=2, space="PSUM") as pstap,
            tc.tile_pool(name="psout", bufs=1, space="PSUM") as psout,
        ):
            posv = posP.rearrange("p (k t) -> p k t", k=6)
            maskv = maskS.rearrange("p (g t) -> p g t", g=GROUP)

            def fetch_chunk(c):
                """Allocate chunk tiles and start their DMAs (pos pairs +
                mask straight into the scat m0 slot)."""
                c0, c1 = cb[c], cb[c + 1]
                tl = c1 - c0
                post = chpos.tile([P, 6 * T_CH], f32, tag="post")
                pov = post[:].rearrange("p (k t) -> p k t", k=6)
                for ax in range(3):
                    nc.sync.dma_start(
                        out=pov[:, 2 * ax:2 * ax + 2, :tl],
                        in_=posv[:, 2 * ax:2 * ax + 2, c0:c1])
                scat = chsc.tile([P, NM * GROUP * T_CH], bf16, tag="scat")
                scv = scat[:].rearrange("p (m g t) -> p m g t", m=NM,
                                        g=GROUP)
                nc.sync.dma_start(out=scv[:, 0, :, :tl],
                                  in_=maskv[:, :, c0:c1])
                return pov, scv

            fetched = {0: fetch_chunk(0)}

            # ---------- constants ----------
            W1T_s = const.tile([NF, HID], f32, tag="w1t")
            W2T_s = const.tile([HID, OUT_DIM], f32, tag="w2t")
            b1_s = const.tile([P, HID], f32, tag="b1")
            b2_s = const.tile([P, OUT_DIM], f32, tag="b2")
            mu_s = const.tile([P, NF], f32, tag="mu")
            gr_s = const.tile([P, 1], f32, tag="gr")
            id_s = const.tile([P, P], f32, tag="id")
            for t, d in ((W1T_s, W1T), (W2T_s, W2T), (b1_s, b1r), (b2_s, b2r),
                         (mu_s, mur), (gr_s, gridc), (id_s, ident)):
                nc.sync.dma_start(out=t[:], in_=d)

            ln2n = const.tile([P, 1], f32, tag="ln2n")
            nc.vector.memset(ln2n[:], -LN2)
            halfc = const.tile([P, 1], f32, tag="halfc")
            nc.vector.memset(halfc[:], 0.5)

            # ---------- filter table (rows 0..P-1; only 0..NB+1 matter) ----
            tg1 = const.tile([P, NF], f32, tag="tg1")
            tg2 = const.tile([P, NF], f32, tag="tg2")
            nc.vector.tensor_tensor(out=tg1[:], in0=gr_s[:].to_broadcast([P, NF]),
                                    in1=mu_s[:], op=OP.subtract)
            nc.scalar.activation(out=tg2[:], in_=tg1[:], func=AF.Square)
            rbf = const.tile([P, NF], f32, tag="rbf")
            nc.scalar.activation(out=rbf[:], in_=tg2[:], func=AF.Exp,
                                 scale=-gamma)

            ptr1 = pspro.tile([NF, P], f32, tag="pro")
            nc.tensor.transpose(out=ptr1[:], in_=rbf[:], identity=id_s[:])
            x0t = const.tile([NF, P], f32, tag="x0t")
            nc.vector.tensor_copy(out=x0t[:], in_=ptr1[:])

            ph = pspro.tile([P, HID], f32, tag="pro")
            nc.tensor.matmul(out=ph[:], lhsT=x0t[:], rhs=W1T_s[:],
                             start=True, stop=True)
            pre1 = const.tile([P, HID], f32, tag="pre1")
            nc.vector.tensor_tensor(out=pre1[:], in0=ph[:], in1=b1_s[:],
                                    op=OP.add)
            e1 = const.tile([P, HID], f32, tag="e1")
            nc.scalar.activation(out=e1[:], in_=pre1[:], func=AF.Exp,
                                 bias=ln2n[:])
            x1 = const.tile([P, HID], f32, tag="x1")
            nc.scalar.activation(out=x1[:], in_=e1[:], func=AF.Ln,
                                 bias=halfc[:])

            ptr2 = pspro.tile([HID, P], f32, tag="pro")
            nc.tensor.transpose(out=ptr2[:], in_=x1[:], identity=id_s[:])
            x1t = const.tile([HID, P], f32, tag="x1t")
            nc.vector.tensor_copy(out=x1t[:], in_=ptr2[:])

            pf = pspro.tile([P, OUT_DIM], f32, tag="pro")
            nc.tensor.matmul(out=pf[:], lhsT=x1t[:], rhs=W2T_s[:],
                             start=True, stop=True)
            pre2 = const.tile([P, OUT_DIM], f32, tag="pre2")
            nc.vector.tensor_tensor(out=pre2[:], in0=pf[:], in1=b2_s[:],
                                    op=OP.add)
            e2 = const.tile([P, OUT_DIM], f32, tag="e2")
            nc.scalar.activation(out=e2[:], in_=pre2[:], func=AF.Exp,
                                 bias=ln2n[:])
            Tf = const.tile([P, OUT_DIM], f32r, tag="Tf")
            nc.scalar.activation(out=Tf[:], in_=e2[:], func=AF.Ln,
                                 bias=halfc[:])

            # shifted copies on bucket rows 0..NB-1
            Tp_s = const.tile([NB, OUT_DIM], f32r, tag="tp")
            Tm_s = const.tile([NB, OUT_DIM], f32r, tag="tm")
            nc.sync.dma_start(out=Tp_s[:], in_=Tf[1:NB + 1, :])
            # row 0 pairs only with bucket 0, which never fires (u >= 1):
            # fill it with T[0] instead of memset (no f32r memset in ISA)
            nc.sync.dma_start(out=Tm_s[0:1, :], in_=Tf[0:1, :])
            nc.sync.dma_start(out=Tm_s[1:NB, :], in_=Tf[0:NB - 1, :])

            # T'_1 = (Tp - Tm)/2 ; T'_2 = (Tp + Tm)/2 - T
            T1b = const.tile([NB, OUT_DIM], f32r, tag="t1b")
            T2b = const.tile([NB, OUT_DIM], f32r, tag="t2b")
            ttmp = const.tile([NB, OUT_DIM], f32r, tag="ttmp")
            nc.vector.tensor_tensor(out=ttmp[:], in0=Tp_s[:], in1=Tm_s[:],
                                    op=OP.subtract)
            nc.vector.tensor_scalar(out=T1b[:], in0=ttmp[:], scalar1=0.5,
                                    scalar2=None, op0=OP.mult)
            nc.vector.tensor_tensor(out=ttmp[:], in0=Tp_s[:], in1=Tm_s[:],
                                    op=OP.add)
            nc.vector.tensor_scalar(out=ttmp[:], in0=ttmp[:], scalar1=0.5,
                                    scalar2=None, op0=OP.mult)
            nc.vector.tensor_tensor(out=T2b[:], in0=ttmp[:], in1=Tf[0:NB, :],
                                    op=OP.subtract)
            Ttabs = [Tf, T1b, T2b]   # lhsT for moments 0,1,2 (rows 0..NB-1)

            lb_tiles = {}
            sc_tiles = {}

            # tile -> (supergroup-local group, k, lastk)
            tinfo = []
            for g in range(NGROUPS):
                cnt = int(tiles_g[g])
                for k in range(cnt):
                    tinfo.append((g % SG_GROUPS, k, cnt - 1))

            for c in range(NCH):
                c0, c1 = cb[c], cb[c + 1]
                tl = c1 - c0

                # ---------- chunk build ----------
                pov, scv = fetched.pop(c)

                w0 = chwork.tile([P, T_CH], f32, tag="w0")
                w1 = chwork.tile([P, T_CH], f32, tag="w1")
                w2 = chwork.tile([P, T_CH], f32, tag="w2")
                wx = [w0, w1, w2]
                # subs on Pool, squares in-place on Act, adds fold into w0
                for ax in range(3):
                    nc.gpsimd.tensor_tensor(
                        out=wx[ax][:, :tl], in0=pov[:, 2 * ax, :tl],
                        in1=pov[:, 2 * ax + 1, :tl], op=OP.subtract)
                for ax in range(3):
                    nc.scalar.activation(out=wx[ax][:, :tl],
                                         in_=wx[ax][:, :tl], func=AF.Square)
                nc.gpsimd.tensor_tensor(out=w0[:, :tl], in0=w0[:, :tl],
                                        in1=w1[:, :tl], op=OP.add)
                nc.gpsimd.tensor_tensor(out=w0[:, :tl], in0=w0[:, :tl],
                                        in1=w2[:, :tl], op=OP.add)
                # u = sqrt(d2)*SCALE, clamped so taps stay in [0, NB)
                nc.scalar.activation(out=w0[:, :tl], in_=w0[:, :tl],
                                     func=AF.Sqrt, scale=SCALE * SCALE)
                nc.vector.tensor_scalar(out=w0[:, :tl], in0=w0[:, :tl],
                                        scalar1=1.001, scalar2=NB - 2.01,
                                        op0=OP.max, op1=OP.min)
                ji = chwork.tile([P, T_CH], i32, tag="ji")
                nc.scalar.copy(out=ji[:, :tl], in_=w0[:, :tl])
                jb = chwork.tile([P, T_CH], bf16, tag="jb")
                nc.scalar.copy(out=jb[:, :tl], in_=ji[:, :tl])
                tb = chwork.tile([P, T_CH], bf16, tag="tb")
                nc.vector.tensor_tensor(out=tb[:, :tl], in0=w0[:, :tl],
                                        in1=jb[:, :tl], op=OP.subtract)

                # one-hot rows (4x DVE mode: all-SBUF packed bf16)
                lb = chlb.tile([P, NB * T_CH], bf16, tag="lb")
                lbv = lb[:].rearrange("p (b t) -> p b t", b=NB)
                for b in range(NB):
                    nc.vector.tensor_scalar(out=lbv[:, b, :tl],
                                            in0=jb[:, :tl],
                                            scalar1=float(b), scalar2=None,
                                            op0=OP.is_equal)

                # moment planes: m1 = mask*t, m2 = m1*t  (2x DVE mode)
                tbv = tb[:, :tl].rearrange("p (o t) -> p o t", o=1)
                nc.vector.tensor_tensor(
                    out=scv[:, 1, :, :tl], in0=scv[:, 0, :, :tl],
                    in1=tbv.to_broadcast([P, GROUP, tl]), op=OP.mult)
                nc.vector.tensor_tensor(
                    out=scv[:, 2, :, :tl], in0=scv[:, 1, :, :tl],
                    in1=tbv.to_broadcast([P, GROUP, tl]), op=OP.mult)

                lb_tiles[c] = (lbv, c0)
                sc_tiles[c] = (scv, c0)

                # prefetch the next chunk before the consume-phase DMAs so
                # its transfers are not head-of-line blocked in the SP queue
                if c + 1 < NCH:
                    fetched[c + 1] = fetch_chunk(c + 1)

                # ---------- consume finished supergroups ----------
                for s in sg_of_chunk[c]:
                    featc = sgp.tile([P, 512], f32, tag="featc")
                    nc.sync.dma_start(out=featc[:],
                                      in_=featT[:, s * 512:(s + 1) * 512])

                    # two PSUM tiles so no matmul output crosses a 2KB
                    # PSUM bank boundary (group blocks of 32B / 64B)
                    tap0 = pstap.tile([NB, SG_GROUPS * GROUP], f32,
                                      tag="tap0")
                    t0v = tap0[:].rearrange("p (g q) -> p g q", q=GROUP)
                    tap12 = pstap.tile([NB, SG_GROUPS * 2 * GROUP], f32,
                                       tag="tap12")
                    t12v = tap12[:].rearrange("p (g m q) -> p g m q", m=2,
                                              q=GROUP)
                    for tt in range(sg_tile0[s], sg_tend[s]):
                        gl, k, lastk = tinfo[tt]
                        cc = c if tt >= cb[c] else c - 1
                        lbv_c, lc0 = lb_tiles[cc]
                        scv_c, sc0 = sc_tiles[cc]
                        nc.tensor.matmul(
                            out=t0v[:, gl, :],
                            lhsT=lbv_c[:, :, tt - lc0],
                            rhs=scv_c[:, 0, :, tt - sc0],
                            start=(k == 0), stop=(k == lastk))
                        nc.tensor.matmul(
                            out=t12v[:, gl, :, :],
                            lhsT=lbv_c[:, :, tt - lc0],
                            rhs=scv_c[:, 1:3, :, tt - sc0],
                            start=(k == 0), stop=(k == lastk))

                    outP = psout.tile([P, 512], f32, tag="outP")
                    for m in range(NM):
                        tsb = sgp.tile([NB, 512], f32r, tag=f"tsb{m}")
                        if m == 0:
                            nc.scalar.copy(out=tsb[:], in_=t0v[:, :, :])
                        else:
                            nc.scalar.copy(out=tsb[:],
                                           in_=t12v[:, :, m - 1, :])
                        nc.tensor.matmul(out=outP[:], lhsT=Ttabs[m][0:NB, :],
                                         rhs=tsb[:], start=(m == 0),
                                         stop=(m == NM - 1))

                    outS = sgp.tile([P, 512], f32, tag="outS")
                    nc.scalar.copy(out=outS[:], in_=outP[:])
                    nc.gpsimd.tensor_tensor(out=outS[:], in0=outS[:],
                                            in1=featc[:], op=OP.mult)
                    nc.sync.dma_start(out=outT[:, s * 512:(s + 1) * 512],
                                      in_=outS[:])

    nc.compile()
    return nc


def kernel(**inputs):
    in_maps, tiles_g, gamma, node_of = _host_prep(inputs)

    key = (tiles_g, round(gamma, 6))
    if key not in _cache:
        _cache[key] = _build(tiles_g, gamma)
    nc = _cache[key]

    from concourse.bass_utils import run_bass_kernel_spmd

    res = run_bass_kernel_spmd(nc, in_maps, core_ids=list(range(NCORES)))

    out = np.empty((N, OUT_DIM), np.float32)
    for c in range(NCORES):
        colnode = node_of[c]
        valid = colnode >= 0
        out[c * NODES_PC + colnode[valid]] = \
            res.results[c]["outT"][:, valid].T
    return out


# revision 21
# speedup vs baseline: 3.4026x; 1.0754x over previous
"""Trainium2 Bass kernel for CFConv (gnn_message_passing).

out[n] = in_node_feat[n] * sum_{e: tgt(e)=n} filt(d_e), where filt(d) is a
function of the scalar edge distance only. The device builds a 64-point
filter table T[b] = filt(b*h) plus the precombined derivative tables
T'_0 = T, T'_1 = (T[b+1]-T[b-1])/2, T'_2 = (T[b+1]+T[b-1])/2 - T[b], so the
quadratic-Lagrange interpolation f(u) = T'_0[j] + t*T'_1[j] + t^2*T'_2[j]
(u = d/h, j = int(u), t = u - j) factors into three per-(node, bucket)
moment histograms H_m[b, n] = sum_{e->n} delta(j_e, b) * t_e^m, accumulated
on the tensor engine, followed by 3 table matmuls per 512 nodes.

The bucket one-hot is built bucket-major (Lb[p, b, t]) with one
tensor_scalar(is_equal, const b) per bucket row, which runs in the DVE
4x perf mode (all-SBUF packed bf16), in 4 big tile-chunks to amortize
per-instruction overheads. The moment rhs is [mask, mask*t, mask*t^2]
with the mask DMA'd straight into its slot and the two products running
in the DVE 2x mode.

Nodes are sharded 8 ways (6250/core); each core processes exactly the
edges targeting its nodes, so no collective is needed. Within a core,
nodes are permuted into degree-balanced groups of 8 (LPT packing) so that
every group fits exactly 2 edge tiles. Host prep does only index work
(shard, group, pad, layout); distances, table, histograms, reduction and
modulation all run on device.
"""

import sys
import numpy as np

sys.path.insert(0, "/opt/trn_rl_repo")

N = 50000
OUT_DIM = 128
NF = 64
HID = 64
NCORES = 8
NODES_PC = N // NCORES           # 6250
GROUP = 8                        # nodes per window group
NODES_PC_PAD = 6656              # 13 * 512
NGROUPS = NODES_PC_PAD // GROUP  # 832
SG_GROUPS = 512 // GROUP         # 64 groups per supergroup (512 nodes)
NSG = NODES_PC_PAD // 512        # 13
NB = 64                          # table buckets
NM = 3                           # moments 1, t, t^2
DMAX = 8.5                       # table covers d in [0, DMAX]
SCALE = (NB - 1) / DMAX
P = 128

_cache = {}


def _lpt_groups(deg):
    """Pack NODES_PC nodes into NGROUPS groups of <= GROUP nodes, balancing
    total degree (greedy LPT). Returns [NGROUPS, GROUP] node ids (-1 pad)."""
    import heapq
    order = np.argsort(-deg, kind="stable")
    heap = [(0, g, 0) for g in range(NGROUPS)]  # (sum, group, count)
    heapq.heapify(heap)
    groups = -np.ones((NGROUPS, GROUP), np.int64)
    deferred = []
    for n in order:
        while True:
            s, g, cnt = heapq.heappop(heap)
            if cnt < GROUP:
                break
            deferred.append((s, g, cnt))
        groups[g, cnt] = n
        heapq.heappush(heap, (s + int(deg[n]), g, cnt + 1))
        for item in deferred:
            heapq.heappush(heap, item)
        deferred.clear()
    return groups


def _host_prep(inputs):
    import ml_dtypes

    pos = np.asarray(inputs["node_pos"], dtype=np.float32)
    ei = np.asarray(inputs["edge_index"])
    src = ei[0].astype(np.int64)
    tgt = ei[1].astype(np.int64)

    core = tgt // NODES_PC
    ln_all = tgt - core * NODES_PC

    per_core = []
    sizes_all = np.zeros((NCORES, NGROUPS), np.int64)
    for c in range(NCORES):
        idx = np.nonzero(core == c)[0]
        ln = ln_all[idx]
        deg = np.bincount(ln, minlength=NODES_PC)
        groups = _lpt_groups(deg)                       # [NGROUPS, GROUP]
        gsum = np.where(groups >= 0, deg[np.maximum(groups, 0)], 0).sum(axis=1)
        # sort groups by size desc so ranked sizes align across cores
        gorder = np.argsort(-gsum, kind="stable")
        groups = groups[gorder]
        gsum = gsum[gorder]
        sizes_all[c] = gsum
        # node -> (group, slot-in-group)
        n2g = np.zeros(NODES_PC, np.int64)
        n2s = np.zeros(NODES_PC, np.int64)
        valid = groups >= 0
        n2g[groups[valid]] = np.repeat(np.arange(NGROUPS), GROUP)[valid.ravel()]
        n2s[groups[valid]] = np.tile(np.arange(GROUP), NGROUPS)[valid.ravel()]
        per_core.append((idx, ln, n2g, n2s, groups))

    tiles_g = np.maximum(1, (sizes_all.max(axis=0) + P - 1) // P)
    gbase = np.zeros(NGROUPS, np.int64)
    np.cumsum(tiles_g[:-1], out=gbase[1:])
    T_TILES = int(tiles_g.sum())
    E_pad = T_TILES * P

    feats = np.asarray(inputs["in_node_feat"], dtype=np.float32)
    in_maps = []
    node_of = []                # per core: output column -> local node (-1 pad)
    for c in range(NCORES):
        idx, ln, n2g, n2s, groups = per_core[c]
        g = n2g[ln]
        slot_in_g = n2s[ln]
        order = np.argsort(g, kind="stable")
        idx = idx[order]
        g = g[order]
        slot_in_g = slot_in_g[order]
        sizes = np.bincount(g, minlength=NGROUPS)
        starts = np.zeros(NGROUPS, np.int64)
        np.cumsum(sizes[:-1], out=starts[1:])
        within = np.arange(len(idx)) - starts[g]
        slot = gbase[g] * P + within

        def plane(vals):
            a = np.zeros(E_pad, np.float32)
            a[slot] = vals
            return np.ascontiguousarray(a.reshape(T_TILES, P).T)

        s_i, t_i = src[idx], tgt[idx]
        m = {}
        # all six planes packed into one [P, 6*T_TILES] tensor, (src,tgt)
        # pairs adjacent so per-axis DMAs can feed the subtractions early
        m["posP"] = np.ascontiguousarray(np.concatenate(
            [plane(pos[s_i, 0]), plane(pos[t_i, 0]), plane(pos[s_i, 1]),
             plane(pos[t_i, 1]), plane(pos[s_i, 2]), plane(pos[t_i, 2])],
            axis=1))

        # mask in group-major layout [P, GROUP, T_TILES]
        msk = np.zeros((E_pad, GROUP), np.float32)
        msk[slot, slot_in_g] = 1.0
        msk = msk.reshape(T_TILES, P, GROUP).transpose(1, 2, 0).reshape(
            P, GROUP * T_TILES)
        m["maskS"] = np.ascontiguousarray(msk).astype(ml_dtypes.bfloat16)

        # feature columns permuted into group order
        colnode = groups.reshape(-1)                    # [NODES_PC_PAD]
        f = np.zeros((P, NODES_PC_PAD), np.float32)
        valid = colnode >= 0
        f[:, valid] = feats[c * NODES_PC + colnode[valid]].T
        m["featT"] = np.ascontiguousarray(f)
        node_of.append(colnode)
        in_maps.append(m)

    lo = float(np.asarray(inputs["lower_bound"]))
    hi = float(np.asarray(inputs["upper_bound"]))
    gamma = float(np.asarray(inputs["gamma"]))
    mu = np.linspace(lo, hi, NF, dtype=np.float32)
    W1 = np.asarray(inputs["W1"], dtype=np.float32)
    W2 = np.asarray(inputs["W2"], dtype=np.float32)
    b1 = np.asarray(inputs["b1"], dtype=np.float32)
    b2 = np.asarray(inputs["b2"], dtype=np.float32)
    consts = {
        "W1T": np.ascontiguousarray(W1.T),
        "W2T": np.ascontiguousarray(W2.T),
        "b1r": np.broadcast_to(b1, (P, HID)).copy(),
        "b2r": np.broadcast_to(b2, (P, OUT_DIM)).copy(),
        "mur": np.broadcast_to(mu, (P, NF)).copy(),
        "gridc": (np.arange(P, dtype=np.float32) / SCALE).reshape(P, 1),
        "ident": np.eye(P, dtype=np.float32),
    }
    for m in in_maps:
        m.update(consts)
    return in_maps, tuple(int(x) for x in tiles_g), gamma, node_of


def _build(tiles_g, gamma):
    from concourse import bacc, mybir
    from concourse.tile import TileContext

    f32 = mybir.dt.float32
    f32r = mybir.dt.float32r
    i32 = mybir.dt.int32
    bf16 = mybir.dt.bfloat16
    AF = mybir.ActivationFunctionType
    OP = mybir.AluOpType
    LN2 = float(np.log(2.0))

    tiles_g = np.asarray(tiles_g, np.int64)
    T_TILES = int(tiles_g.sum())
    gb = np.zeros(NGROUPS + 1, np.int64)
    np.cumsum(tiles_g, out=gb[1:])
    sg_tile0 = [int(gb[SG_GROUPS * s]) for s in range(NSG)]
    sg_tend = [int(gb[SG_GROUPS * (s + 1)]) for s in range(NSG)]

    # chunk bounds: front-loaded so little consumption remains at the tail
    q = T_TILES // 13
    cb = [0, 3 * q + 32, 6 * q + 64, 9 * q + 96, 12 * q, T_TILES]
    NCH = len(cb) - 1
    T_CH = max(cb[i + 1] - cb[i] for i in range(NCH))
    # sgs consumed after each chunk
    sg_of_chunk = [[] for _ in range(NCH)]
    for s in range(NSG):
        for c in range(NCH):
            if sg_tend[s] <= cb[c + 1]:
                sg_of_chunk[c].append(s)
                break

    nc = bacc.Bacc("TRN2", target_bir_lowering=False, debug=False,
                   num_devices=NCORES)

    def din(name, shape, dt=f32):
        return nc.dram_tensor(name, shape, dt, kind="ExternalInput").ap()

    posP = din("posP", [P, 6 * T_TILES])
    maskS = din("maskS", [P, GROUP * T_TILES], bf16)
    featT = din("featT", [P, NODES_PC_PAD])
    W1T = din("W1T", [NF, HID])
    W2T = din("W2T", [HID, OUT_DIM])
    b1r = din("b1r", [P, HID])
    b2r = din("b2r", [P, OUT_DIM])
    mur = din("mur", [P, NF])
    gridc = din("gridc", [P, 1])
    ident = din("ident", [P, P])

    outT = nc.dram_tensor("outT", [P, NODES_PC_PAD], f32,
                          kind="ExternalOutput").ap()

    with TileContext(nc) as tc:
        with (
            tc.tile_pool(name="const", bufs=1) as const,
            tc.tile_pool(name="chpos", bufs=2) as chpos,
            tc.tile_pool(name="chwork", bufs=2) as chwork,
            tc.tile_pool(name="chlb", bufs=2) as chlb,
            tc.tile_pool(name="chsc", bufs=2) as chsc,
            tc.tile_pool(name="sgp", bufs=2) as sgp,
            tc.tile_pool(name="pspro", bufs=1, space="PSUM") as pspro,
            tc.tile_pool(name="pstap", bufs=2, space="PSUM") as pstap,
            tc.tile_pool(name="psout", bufs=1, space="PSUM") as psout,
        ):
            posv = posP.rearrange("p (k t) -> p k t", k=6)
            maskv = maskS.rearrange("p (g t) -> p g t", g=GROUP)

            def fetch_chunk(c):
                """Allocate chunk tiles and start their DMAs (pos pairs +
                mask straight into the scat m0 slot)."""
                c0, c1 = cb[c], cb[c + 1]
                tl = c1 - c0
                post = chpos.tile([P, 6 * T_CH], f32, tag="post")
                pov = post[:].rearrange("p (k t) -> p k t", k=6)
                for ax in range(3):
                    nc.sync.dma_start(
                        out=pov[:, 2 * ax:2 * ax + 2, :tl],
                        in_=posv[:, 2 * ax:2 * ax + 2, c0:c1])
                scat = chsc.tile([P, NM * GROUP * T_CH], bf16, tag="scat")
                scv = scat[:].rearrange("p (m g t) -> p m g t", m=NM,
                                        g=GROUP)
                nc.sync.dma_start(out=scv[:, 0, :, :tl],
                                  in_=maskv[:, :, c0:c1])
                return pov, scv

            fetched = {0: fetch_chunk(0)}

            # ---------- constants ----------
            W1T_s = const.tile([NF, HID], f32, tag="w1t")
            W2T_s = const.tile([HID, OUT_DIM], f32, tag="w2t")
            b1_s = const.tile([P, HID], f32, tag="b1")
            b2_s = const.tile([P, OUT_DIM], f32, tag="b2")
            mu_s = const.tile([P, NF], f32, tag="mu")
            gr_s = const.tile([P, 1], f32, tag="gr")
            id_s = const.tile([P, P], f32, tag="id")
            for t, d in ((W1T_s, W1T), (W2T_s, W2T), (b1_s, b1r), (b2_s, b2r),
                         (mu_s, mur), (gr_s, gridc), (id_s, ident)):
                nc.sync.dma_start(out=t[:], in_=d)

            ln2n = const.tile([P, 1], f32, tag="ln2n")
            nc.vector.memset(ln2n[:], -LN2)
            halfc = const.tile([P, 1], f32, tag="halfc")
            nc.vector.memset(halfc[:], 0.5)

            # ---------- filter table (rows 0..P-1; only 0..NB+1 matter) ----
            tg1 = const.tile([P, NF], f32, tag="tg1")
            tg2 = const.tile([P, NF], f32, tag="tg2")
            nc.vector.tensor_tensor(out=tg1[:], in0=gr_s[:].to_broadcast([P, NF]),
                                    in1=mu_s[:], op=OP.subtract)
            nc.scalar.activation(out=tg2[:], in_=tg1[:], func=AF.Square)
            rbf = const.tile([P, NF], f32, tag="rbf")
            nc.scalar.activation(out=rbf[:], in_=tg2[:], func=AF.Exp,
                                 scale=-gamma)

            ptr1 = pspro.tile([NF, P], f32, tag="pro")
            nc.tensor.transpose(out=ptr1[:], in_=rbf[:], identity=id_s[:])
            x0t = const.tile([NF, P], f32, tag="x0t")
            nc.vector.tensor_copy(out=x0t[:], in_=ptr1[:])

            ph = pspro.tile([P, HID], f32, tag="pro")
            nc.tensor.matmul(out=ph[:], lhsT=x0t[:], rhs=W1T_s[:],
                             start=True, stop=True)
            pre1 = const.tile([P, HID], f32, tag="pre1")
            nc.vector.tensor_tensor(out=pre1[:], in0=ph[:], in1=b1_s[:],
                                    op=OP.add)
            e1 = const.tile([P, HID], f32, tag="e1")
            nc.scalar.activation(out=e1[:], in_=pre1[:], func=AF.Exp,
                                 bias=ln2n[:])
            x1 = const.tile([P, HID], f32, tag="x1")
            nc.scalar.activation(out=x1[:], in_=e1[:], func=AF.Ln,
                                 bias=halfc[:])

            ptr2 = pspro.tile([HID, P], f32, tag="pro")
            nc.tensor.transpose(out=ptr2[:], in_=x1[:], identity=id_s[:])
            x1t = const.tile([HID, P], f32, tag="x1t")
            nc.vector.tensor_copy(out=x1t[:], in_=ptr2[:])

            pf = pspro.tile([P, OUT_DIM], f32, tag="pro")
            nc.tensor.matmul(out=pf[:], lhsT=x1t[:], rhs=W2T_s[:],
                             start=True, stop=True)
            pre2 = const.tile([P, OUT_DIM], f32, tag="pre2")
            nc.vector.tensor_tensor(out=pre2[:], in0=pf[:], in1=b2_s[:],
                                    op=OP.add)
            e2 = const.tile([P, OUT_DIM], f32, tag="e2")
            nc.scalar.activation(out=e2[:], in_=pre2[:], func=AF.Exp,
                                 bias=ln2n[:])
            Tf = const.tile([P, OUT_DIM], f32r, tag="Tf")
            nc.scalar.activation(out=Tf[:], in_=e2[:], func=AF.Ln,
                                 bias=halfc[:])

            # shifted copies on bucket rows 0..NB-1
            Tp_s = const.tile([NB, OUT_DIM], f32r, tag="tp")
            Tm_s = const.tile([NB, OUT_DIM], f32r, tag="tm")
            nc.sync.dma_start(out=Tp_s[:], in_=Tf[1:NB + 1, :])
            # row 0 pairs only with bucket 0, which never fires (u >= 1):
            # fill it with T[0] instead of memset (no f32r memset in ISA)
            nc.sync.dma_start(out=Tm_s[0:1, :], in_=Tf[0:1, :])
            nc.sync.dma_start(out=Tm_s[1:NB, :], in_=Tf[0:NB - 1, :])

            # T'_1 = (Tp - Tm)/2 ; T'_2 = (Tp + Tm)/2 - T
            T1b = const.tile([NB, OUT_DIM], f32r, tag="t1b")
            T2b = const.tile([NB, OUT_DIM], f32r, tag="t2b")
            ttmp = const.tile([NB, OUT_DIM], f32r, tag="ttmp")
            nc.vector.tensor_tensor(out=ttmp[:], in0=Tp_s[:], in1=Tm_s[:],
                                    op=OP.subtract)
            nc.vector.tensor_scalar(out=T1b[:], in0=ttmp[:], scalar1=0.5,
                                    scalar2=None, op0=OP.mult)
            nc.vector.tensor_tensor(out=ttmp[:], in0=Tp_s[:], in1=Tm_s[:],
                                    op=OP.add)
            nc.vector.tensor_scalar(out=ttmp[:], in0=ttmp[:], scalar1=0.5,
                                    scalar2=None, op0=OP.mult)
            nc.vector.tensor_tensor(out=T2b[:], in0=ttmp[:], in1=Tf[0:NB, :],
                                    op=OP.subtract)
            Ttabs = [Tf, T1b, T2b]   # lhsT for moments 0,1,2 (rows 0..NB-1)

            lb_tiles = {}
            sc_tiles = {}

            # tile -> (supergroup-local group, k, lastk)
            tinfo = []
            for g in range(NGROUPS):
                cnt = int(tiles_g[g])
                for k in range(cnt):
                    tinfo.append((g % SG_GROUPS, k, cnt - 1))

            for c in range(NCH):
                c0, c1 = cb[c], cb[c + 1]
                tl = c1 - c0

                # ---------- chunk build ----------
                pov, scv = fetched.pop(c)

                w0 = chwork.tile([P, T_CH], f32, tag="w0")
                w1 = chwork.tile([P, T_CH], f32, tag="w1")
                w2 = chwork.tile([P, T_CH], f32, tag="w2")
                wx = [w0, w1, w2]
                # subs + adds on Pool (chunk 0: DVE, which is idle during
                # the pipeline-fill head), squares in-place on Act, clamp
                # and t on Pool in steady state
                eng_tt = nc.vector if c == 0 else nc.gpsimd
                for ax in range(3):
                    eng_tt.tensor_tensor(
                        out=wx[ax][:, :tl], in0=pov[:, 2 * ax, :tl],
                        in1=pov[:, 2 * ax + 1, :tl], op=OP.subtract)
                for ax in range(3):
                    nc.scalar.activation(out=wx[ax][:, :tl],
                                         in_=wx[ax][:, :tl], func=AF.Square)
                eng_tt.tensor_tensor(out=w0[:, :tl], in0=w0[:, :tl],
                                     in1=w1[:, :tl], op=OP.add)
                eng_tt.tensor_tensor(out=w0[:, :tl], in0=w0[:, :tl],
                                     in1=w2[:, :tl], op=OP.add)
                # u = sqrt(d2)*SCALE, clamped so taps stay in [0, NB)
                nc.scalar.activation(out=w0[:, :tl], in_=w0[:, :tl],
                                     func=AF.Sqrt, scale=SCALE * SCALE)
                eng_cl = nc.vector if c == 0 else nc.gpsimd
                eng_cl.tensor_scalar(out=w0[:, :tl], in0=w0[:, :tl],
                                     scalar1=1.001, scalar2=NB - 2.01,
                                     op0=OP.max, op1=OP.min)
                ji = chwork.tile([P, T_CH], i32, tag="ji")
                nc.scalar.copy(out=ji[:, :tl], in_=w0[:, :tl])
                jb = chwork.tile([P, T_CH], bf16, tag="jb")
                nc.scalar.copy(out=jb[:, :tl], in_=ji[:, :tl])
                tb = chwork.tile([P, T_CH], bf16, tag="tb")
                eng_cl.tensor_tensor(out=tb[:, :tl], in0=w0[:, :tl],
                                     in1=jb[:, :tl], op=OP.subtract)

                # one-hot rows (4x DVE mode: all-SBUF packed bf16)
                lb = chlb.tile([P, NB * T_CH], bf16, tag="lb")
                lbv = lb[:].rearrange("p (b t) -> p b t", b=NB)
                for b in range(NB):
                    nc.vector.tensor_scalar(out=lbv[:, b, :tl],
                                            in0=jb[:, :tl],
                                            scalar1=float(b), scalar2=None,
                                            op0=OP.is_equal)

                # moment planes: m1 = mask*t, m2 = m1*t  (2x DVE mode)
                tbv = tb[:, :tl].rearrange("p (o t) -> p o t", o=1)
                nc.vector.tensor_tensor(
                    out=scv[:, 1, :, :tl], in0=scv[:, 0, :, :tl],
                    in1=tbv.to_broadcast([P, GROUP, tl]), op=OP.mult)
                nc.vector.tensor_tensor(
                    out=scv[:, 2, :, :tl], in0=scv[:, 1, :, :tl],
                    in1=tbv.to_broadcast([P, GROUP, tl]), op=OP.mult)

                lb_tiles[c] = (lbv, c0)
                sc_tiles[c] = (scv, c0)

                # prefetch the next chunk before the consume-phase DMAs so
                # its transfers are not head-of-line blocked in the SP queue
                if c + 1 < NCH:
                    fetched[c + 1] = fetch_chunk(c + 1)

                # ---------- consume finished supergroups ----------
                for s in sg_of_chunk[c]:
                    featc = sgp.tile([P, 512], f32, tag="featc")
                    nc.sync.dma_start(out=featc[:],
                                      in_=featT[:, s * 512:(s + 1) * 512])

                    # two PSUM tiles so no matmul output crosses a 2KB
                    # PSUM bank boundary (group blocks of 32B / 64B)
                    tap0 = pstap.tile([NB, SG_GROUPS * GROUP], f32,
                                      tag="tap0")
                    t0v = tap0[:].rearrange("p (g q) -> p g q", q=GROUP)
                    tap12 = pstap.tile([NB, SG_GROUPS * 2 * GROUP], f32,
                                       tag="tap12")
                    t12v = tap12[:].rearrange("p (g m q) -> p g m q", m=2,
                                              q=GROUP)
                    for tt in range(sg_tile0[s], sg_tend[s]):
                        gl, k, lastk = tinfo[tt]
                        cc = c if tt >= cb[c] else c - 1
                        lbv_c, lc0 = lb_tiles[cc]
                        scv_c, sc0 = sc_tiles[cc]
                        nc.tensor.matmul(
                            out=t0v[:, gl, :],
                            lhsT=lbv_c[:, :, tt - lc0],
                            rhs=scv_c[:, 0, :, tt - sc0],
                            start=(k == 0), stop=(k == lastk))
                        nc.tensor.matmul(
                            out=t12v[:, gl, :, :],
                            lhsT=lbv_c[:, :, tt - lc0],
                            rhs=scv_c[:, 1:3, :, tt - sc0],
                            start=(k == 0), stop=(k == lastk))

                    # evacuate PSUM histograms (kept interleaved; the table
                    # matmuls read strided rhs slices)
                    tsb0 = sgp.tile([NB, SG_GROUPS * GROUP], f32r,
                                    tag="tsb0")
                    nc.scalar.copy(out=tsb0[:], in_=tap0[:])
                    tsb12 = sgp.tile([NB, SG_GROUPS * 2 * GROUP], f32r,
                                     tag="tsb12")
                    nc.scalar.copy(out=tsb12[:], in_=tap12[:])
                    ts12v = tsb12[:].rearrange("p (g m q) -> p g m q", m=2,
                                               q=GROUP)

                    outP = psout.tile([P, 512], f32, tag="outP")
                    nc.tensor.matmul(out=outP[:], lhsT=Ttabs[0][0:NB, :],
                                     rhs=tsb0[:], start=True, stop=False)
                    for m in (1, 2):
                        nc.tensor.matmul(out=outP[:], lhsT=Ttabs[m][0:NB, :],
                                         rhs=ts12v[:, :, m - 1, :],
                                         start=False, stop=(m == 2))

                    outS = sgp.tile([P, 512], f32, tag="outS")
                    nc.scalar.copy(out=outS[:], in_=outP[:])
                    nc.gpsimd.tensor_tensor(out=outS[:], in0=outS[:],
                                            in1=featc[:], op=OP.mult)
                    nc.sync.dma_start(out=outT[:, s * 512:(s + 1) * 512],
                                      in_=outS[:])

    nc.compile()
    return nc


def kernel(**inputs):
    in_maps, tiles_g, gamma, node_of = _host_prep(inputs)

    key = (tiles_g, round(gamma, 6))
    if key not in _cache:
        _cache[key] = _build(tiles_g, gamma)
    nc = _cache[key]

    from concourse.bass_utils import run_bass_kernel_spmd

    res = run_bass_kernel_spmd(nc, in_maps, core_ids=list(range(NCORES)))

    out = np.empty((N, OUT_DIM), np.float32)
    for c in range(NCORES):
        colnode = node_of[c]
        valid = colnode >= 0
        out[c * NODES_PC + colnode[valid]] = \
            res.results[c]["outT"][:, valid].T
    return out
